# revision 22
# baseline (speedup 1.0000x reference)
"""Trainium2 Bass kernel for nn_MetapathRecommender, v3.

Shapes (hardcoded): B=1024, C=8192, P=3, E=64, M=128, H=16, K=8, 8 cores.

v3 changes vs v2 (284us -> target ~100us):
  - The dominant stream pcmT[p] = pc[p].T @ metapaths[p][:, dshard] runs as
    fp8e4m3 x fp8e4m3 with MatmulPerfMode.DoubleRow (2 c-rows per PE cell):
    half the matmul count at ~same per-matmul cost.  pc is host-quantized to
    e4m3 with a power-of-2 scale; the stream is repacked with the DoubleRow
    (Ki=128, Ko=2) interleave.  Stream becomes DMA-bound (~70us of fp8 HBM).
  - Each metapath's d-shard streams in TWO d-passes of 512 so the x-phase of
    pass0 overlaps the pass1 stream, halving the post-stream flush.
  - Rank-1 quantization corrections (pc-quant and mp-quant) are folded into
    the x matmul as two extra contraction rows (outer products), scaled by
    1/8 per core so the ReduceScatter sum reconstitutes them exactly.
  - x partials go psum -> (gpsimd cast DMA) -> collective input directly.
  - qkv is one packed matmul per metapath (Wq pre-scaled by 1/sqrt(K) on the
    host), issued as soon as that metapath's xt lands (overlaps the stream).
  - Attention tail restructured: per-metapath chains split across DVE and
    GpSimd, exp on the scalar engine; output head pipelined over 16 chunks
    with psum->sbuf copies rotating vector/scalar/gpsimd and the 2MB output
    DMA alternating the two HWDGE queues.
"""

import sys
from contextlib import ExitStack

sys.path.insert(0, "/opt/trn_rl_repo")

import numpy as np
import ml_dtypes

import concourse.bass as bass
import concourse.tile as tile
from concourse import mybir
from concourse.bass_utils import run_bass_kernel_spmd

F16 = mybir.dt.float16
F32 = mybir.dt.float32
F8E4 = mybir.dt.float8e4
AF = mybir.ActivationFunctionType
ALU = mybir.AluOpType
DR = mybir.MatmulPerfMode.DoubleRow

B, C, P, E, M, H, K = 1024, 8192, 3, 64, 128, 16, 8
HK = H * K
NCORES = 8
D = C // NCORES          # 1024: d-shard width per core
NB = B // NCORES         # 128: batch rows per core for the tail
NPASS = 2                # d-passes per metapath
DP = D // NPASS          # 512: d extent per pass
NG = 8                   # stream DMA groups per (p, pass); 4 c-pairs each
NPAIR = C // 256         # 32 DoubleRow c-block pairs
NDT = DP // 128          # 4 d tiles per pass
EPS = 1e-12
INV_SQRT_K = 1.0 / float(np.sqrt(np.float32(K)))
PC_SCALE = 256.0         # pc -> e4m3 scale (pow2; |pc|max*256 ~ 26 << 448)

_CACHE = {}


def _split_multi_waits(nc, cap=1):
    """Walrus in this container only accepts `cap` sync-waits per instruction.

    Move extra waits onto freshly inserted NoOps immediately preceding the
    instruction on the same engine."""
    f = nc.m.functions[0]
    nid = 0
    for blk in f.blocks:
        il = blk.instructions
        i = 0
        while i < len(il):
            inst = il[i]
            si = inst.sync_info
            if si is not None and len(si.on_wait) > cap:
                waits = list(si.on_wait)
                extra, keep = waits[:-cap], waits[-cap:]
                for w in extra:
                    nop = mybir.InstNoOp(
                        name=f"I-wsplit-{nid}", engine=inst.engine,
                        sync_info=mybir.SyncInfo(on_wait=[w], on_update=[]))
                    nid += 1
                    il.insert(i, nop)
                    i += 1
                inst.sync_info = mybir.SyncInfo(
                    on_wait=keep, on_update=list(si.on_update))
            i += 1
    return nid


def build_kernel(no_cc=False, repeat=1, ablate=None):
    nc = bass.Bass(num_devices=NCORES)

    # ---- kernel I/O (per-core shards / replicated small tensors) ----
    # mp stream, DoubleRow interleaved: [p, pass, g, ki, pair, ko, d]
    mp_d = nc.dram_tensor("mp_stream", [P, NPASS, NG, 128, 4 * 2 * DP], F8E4,
                          kind="ExternalInput")
    # pc weights, DoubleRow interleaved: [ki, p, pairblk, ko, m]
    pc8_d = nc.dram_tensor("pc8", [128, P * NPAIR * 2 * M], F8E4,
                           kind="ExternalInput")
    poolsT_d = nc.dram_tensor("poolsT_shard", [D, B], F16, kind="ExternalInput")
    # correction outer-product factors: lhs [p][2, M], rhs [p][2, B]
    corrL_d = nc.dram_tensor("corrL", [2 * P, M], F16, kind="ExternalInput")
    corrR_d = nc.dram_tensor("corrR", [2 * P, B], F16, kind="ExternalInput")
    ncT_d = nc.dram_tensor("ncT", [E + 1, C], F16, kind="ExternalInput")
    wqkv_d = nc.dram_tensor("wqkv", [M, 3 * HK], F16, kind="ExternalInput")
    bqkv_d = nc.dram_tensor("bqkv_bc", [NB, 3 * HK], F32, kind="ExternalInput")
    wo_d = nc.dram_tensor("wo", [HK, M], F32, kind="ExternalInput")
    bo3_d = nc.dram_tensor("bo3_col", [M, 1], F32, kind="ExternalInput")
    pk_d = nc.dram_tensor("pool_kernel", [M, E], F32, kind="ExternalInput")
    pb_d = nc.dram_tensor("pool_bias_bc", [NB, E], F32, kind="ExternalInput")
    ident_h_d = nc.dram_tensor("ident_h", [128, 128], F16, kind="ExternalInput")
    ident_f_d = nc.dram_tensor("ident_f", [128, 128], F32, kind="ExternalInput")

    out_d = nc.dram_tensor("out", [NB, C], F16, kind="ExternalOutput")

    with ExitStack() as ctx:
        tc = ctx.enter_context(tile.TileContext(nc, num_cores=NCORES))

        const = ctx.enter_context(tc.tile_pool(name="const", bufs=1))
        dram = ctx.enter_context(tc.tile_pool(name="dram", bufs=1, space="DRAM"))

        # ---------- load constants (stream-critical first, on scalar q) ----
        pc8_sb = const.tile([128, P, NPAIR, 2, M], F8E4, name="pc8")
        for p in range(P):
            nc.scalar.dma_start(
                pc8_sb[:, p, :, :, :].rearrange("q a b m -> q (a b m)"),
                pc8_d[:, p * NPAIR * 2 * M:(p + 1) * NPAIR * 2 * M])
        poolsT_sb = const.tile([128, D // 128, B], F16)  # (d%128, dtile, b)
        nc.scalar.dma_start(
            poolsT_sb[:], poolsT_d.ap().rearrange("(t p) b -> p t b", p=128))
        corrL_sb = [const.tile([2, M], F16, name=f"corrL{p}") for p in range(P)]
        corrR_sb = [const.tile([2, B], F16, name=f"corrR{p}") for p in range(P)]
        for p in range(P):
            nc.scalar.dma_start(corrL_sb[p][:], corrL_d[2 * p:2 * p + 2, :])
            nc.scalar.dma_start(corrR_sb[p][:], corrR_d[2 * p:2 * p + 2, :])
        wqkv_sb = const.tile([M, 3 * HK], F16)
        nc.scalar.dma_start(wqkv_sb[:], wqkv_d[:, :])
        bqkv_sb = const.tile([NB, 3 * HK], F32)
        nc.scalar.dma_start(bqkv_sb[:], bqkv_d[:, :])
        wo_sb = const.tile([HK, M], F32)
        nc.scalar.dma_start(wo_sb[:], wo_d[:, :])
        bo3_sb = const.tile([M, 1], F32)
        nc.scalar.dma_start(bo3_sb[:], bo3_d[:, :])
        pk_sb = const.tile([M, E], F32)
        nc.scalar.dma_start(pk_sb[:], pk_d[:, :])
        pb_sb = const.tile([NB, E], F32)
        nc.scalar.dma_start(pb_sb[:], pb_d[:, :])
        ident_h = const.tile([128, 128], F16)
        nc.scalar.dma_start(ident_h[:], ident_h_d[:, :])
        ident_f = const.tile([128, 128], F32)
        nc.scalar.dma_start(ident_f[:], ident_f_d[:, :])
        eps_sb = const.tile([128, 1], F32)
        nc.vector.memset(eps_sb[:], 4.0 * EPS)
        ncT_sb = const.tile([E + 1, C], F16)
        nc.scalar.dma_start(ncT_sb[:], ncT_d[:, :])
        # npT row E is the constant 0.5 that turns npT.T @ ncT_aug into
        # (cos + 1)/2 directly (ncT_aug's last row is all ones)
        npT = const.tile([E + 1, NB], F16, name="npT")
        nc.vector.memset(npT[E:E + 1, :], 0.5)

        for _rep in range(repeat):
            # ---------- stream + x partials + per-p RS + qkv ----------
            stream_ctx = ExitStack()
            mm_psum = stream_ctx.enter_context(
                tc.tile_pool(name="mm_psum", bufs=3, space="PSUM"))     # 3 banks
            trx_psum = stream_ctx.enter_context(
                tc.tile_pool(name="trx_psum", bufs=1, space="PSUM"))    # 1 bank
            xq_psum = stream_ctx.enter_context(
                tc.tile_pool(name="xq_psum", bufs=1, space="PSUM"))     # 2+1+1 banks
            mp_pool = stream_ctx.enter_context(tc.tile_pool(name="mp", bufs=16))
            pcm_pool = stream_ctx.enter_context(tc.tile_pool(name="pcm", bufs=4))

            cc_in = [dram.tile([NCORES, M, NB], F16, name=f"cci{p}") for p in range(P)]
            cc_out = [dram.tile([M, NB], F16, name=f"cco{p}") for p in range(P)]
            xt = [const.tile([M, NB], F16, name=f"xt{p}") for p in range(P)]
            QKV = const.tile([NB, P, 3 * HK], F32, name="QKV")
            sink = const.tile([128, 4], F16, name="sink", tag="sink") if ablate else None

            # attention-chain tiles ([b, p, h, q2] layouts; contiguous X reduces)
            Q = QKV[:].rearrange("b p (t h k) -> b p t h k", t=3, k=K)
            prod = const.tile([NB, P, H, P, K], F32, name="prod")
            S = const.tile([NB, P, H, P], F32, name="S")
            mx = const.tile([NB, P, H], F32, name="mx")
            Et = const.tile([NB, P, H, P], F32, name="Et")
            den = const.tile([NB, P, H], F32, name="den")
            rden = const.tile([NB, P, H], F32, name="rden")
            ATT = const.tile([NB, P, H, P], F32, name="ATT")
            prod2 = const.tile([NB, P, H, K, P], F32, name="prod2")
            O = const.tile([NB, P, HK], F32, name="O")
            oT = [const.tile([HK, NB], F32, name=f"oT{p}") for p in range(P)]

            def attn_chain(p):
                """softmax(q.kT).v for metapath p on DVE (+ exp on ACT),
                ending with O[p] transposed into oT[p].  Issued eagerly from
                finish_p so p0/p1 run under the remaining stream."""
                ve = nc.vector
                qv = Q[:, p, 0][:, :, None, :]                 # [b, h, 1, k]
                kv = Q[:, :, 1].rearrange("b q h k -> b h q k")
                qb, kb = bass.broadcast_tensor_aps(qv, kv)
                ve.tensor_tensor(prod[:, p], qb, kb, ALU.mult)
                ve.tensor_reduce(S[:, p], prod[:, p],
                                 axis=mybir.AxisListType.X, op=ALU.add)
                ve.tensor_reduce(mx[:, p], S[:, p],
                                 axis=mybir.AxisListType.X, op=ALU.max)
                sb_, mb_ = bass.broadcast_tensor_aps(S[:, p], mx[:, p][:, :, None])
                ve.tensor_tensor(Et[:, p], sb_, mb_, ALU.subtract)
                nc.scalar.activation(Et[:, p].rearrange("b h q -> b (h q)"),
                                     Et[:, p].rearrange("b h q -> b (h q)"),
                                     AF.Exp)
                ve.tensor_reduce(den[:, p], Et[:, p],
                                 axis=mybir.AxisListType.X, op=ALU.add)
                ve.reciprocal(rden[:, p], den[:, p])
                eb_, rb_ = bass.broadcast_tensor_aps(Et[:, p], rden[:, p][:, :, None])
                ve.tensor_tensor(ATT[:, p], eb_, rb_, ALU.mult)
                av = ATT[:, p][:, :, None, :]                  # [b, h, 1, q2]
                vv = Q[:, :, 2].rearrange("b q h k -> b h k q")
                ab_, vb_ = bass.broadcast_tensor_aps(av, vv)
                ve.tensor_tensor(prod2[:, p], ab_, vb_, ALU.mult)
                ve.tensor_reduce(O[:, p].rearrange("b (h k) -> b h k", k=K),
                                 prod2[:, p],
                                 axis=mybir.AxisListType.X, op=ALU.add)
                oT_ps = xq_psum.tile([NB, HK], F32, tag="otps")
                nc.tensor.transpose(oT_ps[:], O[:, p, :], ident_f[:])
                nc.vector.tensor_copy(oT[p][:], oT_ps[:])

            x_ps = {}

            def x_phase(p, pas, ps):
                """psum pcmT (pass) -> f16 -> transpose -> x partial MMs."""
                cp_eng = (nc.vector, nc.scalar)[pas]
                pcmT = pcm_pool.tile([128, DP], F16, tag="pcmT")
                if cp_eng is nc.scalar:
                    nc.scalar.activation(pcmT[:], ps[:], AF.Copy)
                else:
                    nc.vector.tensor_copy(pcmT[:], ps[:])
                trp = trx_psum.tile([128, DP], F16, tag="trx")
                for dt in range(NDT):
                    nc.tensor.transpose(trp[:, dt * 128:(dt + 1) * 128],
                                        pcmT[:, dt * 128:(dt + 1) * 128],
                                        ident_h[:])
                pcm_dd = pcm_pool.tile([128, DP], F16, tag="pcmd")
                if cp_eng is nc.scalar:
                    nc.vector.tensor_copy(pcm_dd[:], trp[:])
                else:
                    nc.scalar.activation(pcm_dd[:], trp[:], AF.Copy)
                xp = x_ps[p]
                for half in range(2):
                    hb = slice(half * 512, (half + 1) * 512)
                    for dt in range(NDT):
                        nc.tensor.matmul(
                            xp[:, hb],
                            lhsT=pcm_dd[:, dt * 128:(dt + 1) * 128],
                            rhs=poolsT_sb[:, pas * NDT + dt, hb],
                            start=(pas == 0 and dt == 0), stop=False)
                    if pas == NPASS - 1:
                        # rank-2 quantization correction as 2 extra rows
                        nc.tensor.matmul(
                            xp[:, hb],
                            lhsT=corrL_sb[p][:, :],
                            rhs=corrR_sb[p][:, hb],
                            start=False, stop=True)

            def finish_p(p):
                """x psum -> sbuf f16 -> cc_in -> RS -> xt -> packed qkv."""
                xp = x_ps[p]
                xsb = const.tile([M, B], F16, name=f"xsb{p}")
                nc.vector.tensor_copy(xsb[:, 0:512], xp[:, 0:512])
                nc.scalar.activation(xsb[:, 512:B], xp[:, 512:B], AF.Copy)
                nc.gpsimd.dma_start(
                    cc_in[p].rearrange("j m b -> m j b"),
                    xsb[:].rearrange("m (j b) -> m j b", j=NCORES))
                if no_cc:
                    nc.gpsimd.dma_start(cc_out[p][:, :], cc_in[p][0, :, :])
                else:
                    nc.gpsimd.collective_compute(
                        "ReduceScatter", ALU.add,
                        replica_groups=[list(range(NCORES))],
                        ins=[cc_in[p][:, :, :]],
                        outs=[cc_out[p][:, :]],
                    )
                nc.gpsimd.dma_start(xt[p][:], cc_out[p][:, :])
                if ablate == "notail":
                    nc.vector.tensor_copy(sink[:], xt[p][:, :4])

            def qkv_chain(p):
                """packed qkv matmul for metapath p (chain runs at the tail:
                scores/values mix ALL metapaths, so it's gated on qkv(P-1))."""
                qp = xq_psum.tile([NB, 3 * HK], F32, tag="qkv")
                nc.tensor.matmul(qp[:], lhsT=xt[p][:], rhs=wqkv_sb[:],
                                 start=True, stop=True)
                nc.vector.tensor_add(QKV[:, p, :], qp[:], bqkv_sb[:])
                if p == P - 1:
                    for pp in range(P):
                        attn_chain(pp)

            # stream loop: p-pass pipeline with x_phase/finish interleaved
            pend = []
            for p in range(P):
                x_ps[p] = xq_psum.tile([M, B], F32, tag="xps", name="xps")
                for pas in range(NPASS):
                    ps = mm_psum.tile([128, DP], F32, tag="mmps")
                    for g in range(NG):
                        mp_t = mp_pool.tile([128, 4, 2, DP], F8E4, tag="mpt")
                        nc.sync.dma_start(
                            mp_t[:].rearrange("q a b d -> q (a b d)"),
                            mp_d[p, pas, g, :, :])
                        for pr in range(4):
                            pair = g * 4 + pr
                            nc.tensor.matmul(
                                ps[:],
                                lhsT=pc8_sb[:, p, pair, :, :],
                                rhs=mp_t[:, pr, :, :],
                                start=(pair == 0), stop=(pair == NPAIR - 1),
                                perf_mode=DR)
                    if ablate == "nox":
                        nc.vector.tensor_copy(sink[:], ps[:, :4])
                        continue
                    # finish_p(p-1) reads the shared x psum; it must be
                    # issued before x_phase(p, 0) starts overwriting it
                    if pas == NPASS - 1 and p >= 1:
                        finish_p(p - 1)
                    # flush previous pending x-phase AFTER issuing this
                    # pass's stream so PE work interleaves
                    pend.append((p, pas, ps))
                    if len(pend) >= 2:
                        x_phase(*pend.pop(0))
                    # qkv+softmax for p-1 after x_phase's copies so its
                    # RS-wait doesn't head-block the DVE/ACT queues
                    if pas == NPASS - 1 and p >= 1 and ablate is None:
                        qkv_chain(p - 1)
            if ablate != "nox":
                while pend:
                    x_phase(*pend.pop(0))
                finish_p(P - 1)
                if ablate is None:
                    qkv_chain(P - 1)

            stream_ctx.close()

            if ablate is not None:
                if ablate == "nox":
                    nc.gpsimd.dma_start(out_d[:, :4], sink[:NB, :])
                else:
                    nc.gpsimd.dma_start(out_d[:, :4], sink[:NB, :])
                continue

            # ---------- tail: attended projection + pool head + output ----------
            tail_ctx = ExitStack()
            tail = tail_ctx.enter_context(tc.tile_pool(name="tail", bufs=1))
            tail_ps = tail_ctx.enter_context(
                tc.tile_pool(name="tail_ps", bufs=1, space="PSUM"))

            # attendedT[m, b] = sum_p Wo.T @ O[p].T  (wo stationary)
            att_ps = tail_ps.tile([M, NB], F32, tag="attps")
            for p in range(P):
                nc.tensor.matmul(att_ps[:], lhsT=wo_sb[:], rhs=oT[p][:],
                                 start=(p == 0), stop=(p == P - 1))

            # preT (m, b) = attendedT + 3*bo ; pool head
            preT = const.tile([M, NB], F32, name="preT")
            nc.scalar.activation(preT[:], att_ps[:], AF.Identity, bias=bo3_sb[:])
            pe_ps = tail_ps.tile([NB, E], F32, tag="peps")
            nc.tensor.matmul(pe_ps[:], lhsT=preT[:], rhs=pk_sb[:],
                             start=True, stop=True)
            pe = const.tile([NB, E], F32, name="pe")
            nc.vector.tensor_add(pe[:], pe_ps[:], pb_sb[:])
            sq2 = const.tile([NB, E], F32, name="sq2")
            ss2 = const.tile([NB, 1], F32, name="ss2")
            nc.scalar.activation(sq2[:], pe[:], AF.Square, accum_out=ss2[:])
            # 2*sqrt(ss + EPS) via Sqrt(4*ss + 4*EPS), so npn = pe/(2*||pe||)
            nc.scalar.activation(ss2[:], ss2[:], AF.Sqrt, bias=eps_sb[:NB, :],
                                 scale=4.0)
            rr = const.tile([NB, 1], F32, name="rr")
            nc.vector.reciprocal(rr[:], ss2[:])
            npn = const.tile([NB, E], F32, name="npn")
            nc.scalar.activation(npn[:], pe[:], AF.Copy, scale=rr[:])
            npT_ps = tail_ps.tile([E, NB], F32, tag="npps")
            nc.tensor.transpose(npT_ps[:], npn[:], ident_f[:])
            nc.vector.tensor_copy(npT[:E, :], npT_ps[:])

            # final: out = npT_aug.T @ ncT_aug = (cos+1)/2 directly
            dq_engs = (nc.scalar, nc.sync, nc.gpsimd)
            for ch in range(C // 512):
                fp = tail_ps.tile([NB, 512], F32, tag="fin", bufs=3)
                nc.tensor.matmul(fp[:], lhsT=npT[:],
                                 rhs=ncT_sb[:, ch * 512:(ch + 1) * 512],
                                 start=True, stop=True)
                och = tail.tile([NB, 512], F16, tag="och", bufs=6)
                if ch % 2:
                    nc.scalar.activation(och[:], fp[:], AF.Copy)
                else:
                    nc.vector.tensor_copy(och[:], fp[:])
                dq_engs[ch % 3].dma_start(out_d[:, ch * 512:(ch + 1) * 512],
                                          och[:])
            tail_ctx.close()

    _split_multi_waits(nc)
    return nc


def _silu64(x):
    return x / (1.0 + np.exp(-x))


def _prep_inputs(inputs):
    f8 = ml_dtypes.float8_e4m3
    h16 = np.float16
    pools = np.asarray(inputs["pools"], np.float32)
    metapaths = np.asarray(inputs["metapaths"], np.float32)
    ce = np.asarray(inputs["card_embeddings"], np.float32)
    mpk = np.asarray(inputs["mp_kernels"], np.float32)
    mpb = np.asarray(inputs["mp_biases"], np.float32)

    # ---- host: pc[p] (C, M) in f64 -> e4m3 (scaled) + DoubleRow repack ----
    ce64 = ce.astype(np.float64)
    pc = np.empty((P, C, M), np.float64)
    for p in range(P):
        pc[p] = _silu64(ce64 @ mpk[p].astype(np.float64)
                        + mpb[p, :, 0].astype(np.float64)[None, :])
    pc8 = (pc * PC_SCALE).astype(f8)                      # (P, C, M)
    pcq = pc8.astype(np.float64) / PC_SCALE
    # pc8 repack: [ki, p, pair, ko, m], c = (2*pair + ko)*128 + ki
    pc8_r = (pc8.reshape(P, NPAIR, 2, 128, M)
             .transpose(3, 0, 1, 2, 4))                   # ki, p, pair, ko, m
    pc8_flat = np.ascontiguousarray(pc8_r.reshape(128, P * NPAIR * 2 * M))

    # ---- host: mp -> e4m3 ----
    mp8 = metapaths.astype(f8)                            # (P, C, C)
    mpq = mp8.astype(np.float64)

    # ---- rank-1 corrections (pc-quant and mp-quant), as outer products ----
    # x_err = sum_c dpc[c,m] * z[b,c] + sum_c pcq[c,m] * dz[b,c]
    #   dpc approx: Sdelta[m] * w[b],  w = pools @ mean_c(mp)
    #   dmp approx: u[m] * w2[b],      u = sum_c pcq,  w2 = pools @ mean_c(dmp)
    corrL = np.empty((2 * P, M), np.float64)
    corrR = np.empty((2 * P, B), np.float64)
    pools64 = pools.astype(np.float64)
    for p in range(P):
        sdelta = (pc[p] - pcq[p]).sum(axis=0)             # (M,)
        w = pools64 @ metapaths[p].astype(np.float64).mean(axis=0)   # (B,)
        u = pcq[p].sum(axis=0)                            # (M,)
        w2 = pools64 @ (metapaths[p].astype(np.float64) - mpq[p]).mean(axis=0)
        # balance the factor magnitudes for f16 storage
        for row, (lv, rv) in enumerate(((sdelta, w), (u, w2))):
            s = np.sqrt((np.abs(lv).max() + 1e-30) / (np.abs(rv).max() + 1e-30))
            corrL[2 * p + row] = lv / s
            corrR[2 * p + row] = rv * s
    corrR /= NCORES   # each core contributes corr/8; RS sums to corr

    # ---- host: normalized cards (E+1, C) fp16, ones row appended ----
    nrm = np.sqrt(np.maximum((ce.astype(np.float64) ** 2).sum(axis=1), EPS))
    ncT = (ce.astype(np.float64) / nrm[:, None]).T.astype(h16)
    ncT = np.ascontiguousarray(np.concatenate(
        [ncT, np.ones((1, C), h16)], axis=0))

    # 1/PC_SCALE folded into poolsT: x = (pc*256).T @ mp @ (pools/256)
    poolsT = np.ascontiguousarray(pools.T / PC_SCALE).astype(h16)

    # ---- packed qkv weights; 1/sqrt(K) folded into Wq (and bq) ----
    wq = np.asarray(inputs["Wq"], np.float32).reshape(M, HK) * INV_SQRT_K
    wk = np.asarray(inputs["Wk"], np.float32).reshape(M, HK)
    wv = np.asarray(inputs["Wv"], np.float32).reshape(M, HK)
    wqkv = np.concatenate([wq, wk, wv], axis=1)           # (M, 384)
    bq = np.asarray(inputs["bq"], np.float32).reshape(HK) * INV_SQRT_K
    bk = np.asarray(inputs["bk"], np.float32).reshape(HK)
    bv = np.asarray(inputs["bv"], np.float32).reshape(HK)
    bqkv = np.concatenate([bq, bk, bv])

    com = {
        "pc8": pc8_flat,
        "corrL": np.ascontiguousarray(corrL.astype(h16)),
        "ncT": ncT,
        "wqkv": np.ascontiguousarray(wqkv.astype(h16)),
        "bqkv_bc": np.ascontiguousarray(np.broadcast_to(
            bqkv.reshape(1, 3 * HK), (NB, 3 * HK)).astype(np.float32)),
        "wo": np.ascontiguousarray(np.asarray(inputs["Wo"], np.float32).reshape(HK, M)),
        "bo3_col": np.ascontiguousarray(
            (P * np.asarray(inputs["bo"], np.float32)).reshape(M, 1)),
        "pool_kernel": np.ascontiguousarray(np.asarray(inputs["pool_kernel"], np.float32)),
        "pool_bias_bc": np.ascontiguousarray(np.broadcast_to(
            np.asarray(inputs["pool_bias"], np.float32).reshape(1, E), (NB, E))),
        "ident_h": np.eye(128, dtype=h16),
        "ident_f": np.eye(128, dtype=np.float32),
    }
    in_maps = []
    for i in range(NCORES):
        m = dict(com)
        # mp stream repack: [p, pass, g, ki, pair, ko, d]
        # c = (2*(g*4+pair) + ko)*128 + ki ; d = i*D + pass*DP + dlocal
        sl = mp8[:, :, i * D:(i + 1) * D]                 # (P, C, D)
        st = sl.reshape(P, NG, 4, 2, 128, NPASS, DP)      # c-major split
        st = st.transpose(0, 5, 1, 4, 2, 3, 6)            # p, pass, g, ki, pair, ko, d
        m["mp_stream"] = np.ascontiguousarray(
            st.reshape(P, NPASS, NG, 128, 4 * 2 * DP))
        m["poolsT_shard"] = np.ascontiguousarray(poolsT[i * D:(i + 1) * D, :])
        m["corrR"] = np.ascontiguousarray(corrR.astype(h16))
        in_maps.append(m)
    return in_maps


def kernel(**inputs) -> np.ndarray:
    if "nc" not in _CACHE:
        _CACHE["nc"] = build_kernel()
    nc = _CACHE["nc"]
    in_maps = _prep_inputs(inputs)
    res = run_bass_kernel_spmd(nc, in_maps, core_ids=list(range(NCORES)))
    outs = [np.asarray(res.results[i]["out"]).astype(np.float32)
            for i in range(NCORES)]
    return np.concatenate(outs, axis=0)


if __name__ == "__main__":
    nc = build_kernel()
    print("kernel built OK")


# revision 28
# speedup vs baseline: 1.4498x; 1.4498x over previous
"""Trainium2 Bass kernel for nn_MetapathRecommender, v3.

Shapes (hardcoded): B=1024, C=8192, P=3, E=64, M=128, H=16, K=8, 8 cores.

v3 changes vs v2 (284us -> target ~100us):
  - The dominant stream pcmT[p] = pc[p].T @ metapaths[p][:, dshard] runs as
    fp8e4m3 x fp8e4m3 with MatmulPerfMode.DoubleRow (2 c-rows per PE cell):
    half the matmul count at ~same per-matmul cost.  pc is host-quantized to
    e4m3 with a power-of-2 scale; the stream is repacked with the DoubleRow
    (Ki=128, Ko=2) interleave.  Stream becomes DMA-bound (~70us of fp8 HBM).
  - Each metapath's d-shard streams in TWO d-passes of 512 so the x-phase of
    pass0 overlaps the pass1 stream, halving the post-stream flush.
  - Rank-1 quantization corrections (pc-quant and mp-quant) are folded into
    the x matmul as two extra contraction rows (outer products), scaled by
    1/8 per core so the ReduceScatter sum reconstitutes them exactly.
  - x partials go psum -> (gpsimd cast DMA) -> collective input directly.
  - qkv is one packed matmul per metapath (Wq pre-scaled by 1/sqrt(K) on the
    host), issued as soon as that metapath's xt lands (overlaps the stream).
  - Attention tail restructured: per-metapath chains split across DVE and
    GpSimd, exp on the scalar engine; output head pipelined over 16 chunks
    with psum->sbuf copies rotating vector/scalar/gpsimd and the 2MB output
    DMA alternating the two HWDGE queues.
"""

import sys
from contextlib import ExitStack

sys.path.insert(0, "/opt/trn_rl_repo")

import numpy as np
import ml_dtypes

import concourse.bass as bass
import concourse.tile as tile
from concourse import mybir
from concourse.bass_utils import run_bass_kernel_spmd

F16 = mybir.dt.float16
F32 = mybir.dt.float32
F8E4 = mybir.dt.float8e4
AF = mybir.ActivationFunctionType
ALU = mybir.AluOpType
DR = mybir.MatmulPerfMode.DoubleRow

B, C, P, E, M, H, K = 1024, 8192, 3, 64, 128, 16, 8
HK = H * K
NCORES = 8
D = C // NCORES          # 1024: d-shard width per core
NB = B // NCORES         # 128: batch rows per core for the tail
NPASS = 2                # d-passes per metapath
DP = D // NPASS          # 512: d extent per pass
NG = 8                   # stream DMA groups per (p, pass); 4 c-pairs each
NPAIR = C // 256         # 32 DoubleRow c-block pairs
NDT = DP // 128          # 4 d tiles per pass
EPS = 1e-12
INV_SQRT_K = 1.0 / float(np.sqrt(np.float32(K)))
PC_SCALE = 256.0         # pc -> e4m3 scale (pow2; |pc|max*256 ~ 26 << 448)

_CACHE = {}


def _split_multi_waits(nc, cap=1):
    """Walrus in this container only accepts `cap` sync-waits per instruction.

    Move extra waits onto freshly inserted NoOps immediately preceding the
    instruction on the same engine."""
    f = nc.m.functions[0]
    nid = 0
    for blk in f.blocks:
        il = blk.instructions
        i = 0
        while i < len(il):
            inst = il[i]
            si = inst.sync_info
            if si is not None and len(si.on_wait) > cap:
                waits = list(si.on_wait)
                extra, keep = waits[:-cap], waits[-cap:]
                for w in extra:
                    nop = mybir.InstNoOp(
                        name=f"I-wsplit-{nid}", engine=inst.engine,
                        sync_info=mybir.SyncInfo(on_wait=[w], on_update=[]))
                    nid += 1
                    il.insert(i, nop)
                    i += 1
                inst.sync_info = mybir.SyncInfo(
                    on_wait=keep, on_update=list(si.on_update))
            i += 1
    return nid


def build_kernel(no_cc=False, repeat=1, ablate=None):
    nc = bass.Bass(num_devices=NCORES)

    # ---- kernel I/O (per-core shards / replicated small tensors) ----
    # mp stream, DoubleRow interleaved: [p, pass, g, ki, pair, ko, d]
    mp_d = nc.dram_tensor("mp_stream", [P, NPASS, NG, 128, 4 * 2 * DP], F8E4,
                          kind="ExternalInput")
    # pc weights, DoubleRow interleaved: [ki, p, pairblk, ko, m]
    pc8_d = nc.dram_tensor("pc8", [128, P * NPAIR * 2 * M], F8E4,
                           kind="ExternalInput")
    poolsT_d = nc.dram_tensor("poolsT_shard", [D, B], F16, kind="ExternalInput")
    # correction outer-product factors: lhs [p][2, M], rhs [p][2, B]
    corrL_d = nc.dram_tensor("corrL", [2 * P, M], F16, kind="ExternalInput")
    corrR_d = nc.dram_tensor("corrR", [2 * P, B], F16, kind="ExternalInput")
    ncT_d = nc.dram_tensor("ncT", [E + 1, C], F16, kind="ExternalInput")
    wqkv_d = nc.dram_tensor("wqkv", [M, 3 * HK], F16, kind="ExternalInput")
    bqkv_d = nc.dram_tensor("bqkv_bc", [NB, 3 * HK], F32, kind="ExternalInput")
    wo_d = nc.dram_tensor("wo", [HK, M], F32, kind="ExternalInput")
    bo3_d = nc.dram_tensor("bo3_col", [M, 1], F32, kind="ExternalInput")
    pk_d = nc.dram_tensor("pool_kernel", [M, E], F32, kind="ExternalInput")
    pb_d = nc.dram_tensor("pool_bias_bc", [NB, E], F32, kind="ExternalInput")
    ident_h_d = nc.dram_tensor("ident_h", [128, 128], F16, kind="ExternalInput")
    ident_f_d = nc.dram_tensor("ident_f", [128, 128], F32, kind="ExternalInput")

    out_d = nc.dram_tensor("out", [NB, C], F16, kind="ExternalOutput")

    with ExitStack() as ctx:
        tc = ctx.enter_context(tile.TileContext(nc, num_cores=NCORES))

        const = ctx.enter_context(tc.tile_pool(name="const", bufs=1))
        dram = ctx.enter_context(tc.tile_pool(name="dram", bufs=1, space="DRAM"))

        # ---------- load constants (stream-critical first, on scalar q) ----
        pc8_sb = const.tile([128, P, NPAIR, 2, M], F8E4, name="pc8")
        for p in range(P):
            nc.scalar.dma_start(
                pc8_sb[:, p, :, :, :].rearrange("q a b m -> q (a b m)"),
                pc8_d[:, p * NPAIR * 2 * M:(p + 1) * NPAIR * 2 * M])
        poolsT_sb = const.tile([128, D // 128, B], F16)  # (d%128, dtile, b)
        nc.scalar.dma_start(
            poolsT_sb[:], poolsT_d.ap().rearrange("(t p) b -> p t b", p=128))
        corrL_sb = [const.tile([2, M], F16, name=f"corrL{p}") for p in range(P)]
        corrR_sb = [const.tile([2, B], F16, name=f"corrR{p}") for p in range(P)]
        for p in range(P):
            nc.scalar.dma_start(corrL_sb[p][:], corrL_d[2 * p:2 * p + 2, :])
            nc.scalar.dma_start(corrR_sb[p][:], corrR_d[2 * p:2 * p + 2, :])
        wqkv_sb = const.tile([M, 3 * HK], F16)
        nc.scalar.dma_start(wqkv_sb[:], wqkv_d[:, :])
        bqkv_sb = const.tile([NB, 3 * HK], F32)
        nc.scalar.dma_start(bqkv_sb[:], bqkv_d[:, :])
        wo_sb = const.tile([HK, M], F32)
        nc.scalar.dma_start(wo_sb[:], wo_d[:, :])
        bo3_sb = const.tile([M, 1], F32)
        nc.scalar.dma_start(bo3_sb[:], bo3_d[:, :])
        pk_sb = const.tile([M, E], F32)
        nc.scalar.dma_start(pk_sb[:], pk_d[:, :])
        pb_sb = const.tile([NB, E], F32)
        nc.scalar.dma_start(pb_sb[:], pb_d[:, :])
        ident_h = const.tile([128, 128], F16)
        nc.scalar.dma_start(ident_h[:], ident_h_d[:, :])
        ident_f = const.tile([128, 128], F32)
        nc.scalar.dma_start(ident_f[:], ident_f_d[:, :])
        eps_sb = const.tile([128, 1], F32)
        nc.vector.memset(eps_sb[:], 4.0 * EPS)
        ncT_sb = const.tile([E + 1, C], F16)
        nc.scalar.dma_start(ncT_sb[:], ncT_d[:, :])
        # npT row E is the constant 0.5 that turns npT.T @ ncT_aug into
        # (cos + 1)/2 directly (ncT_aug's last row is all ones)
        npT = const.tile([E + 1, NB], F16, name="npT")
        nc.vector.memset(npT[E:E + 1, :], 0.5)

        # ---------- persistent pools (never close: a per-rep close/open
        # would alias the next rep's stream tiles onto this rep's tail
        # tiles and serialize the repeat pipeline on the tail's last DMA)
        # PSUM budget: mm 2 + trx 1 + xps 2 + sm 3 = 8 banks exactly.
        mm_psum = ctx.enter_context(
            tc.tile_pool(name="mm_psum", bufs=2, space="PSUM"))
        trx_psum = ctx.enter_context(
            tc.tile_pool(name="trx_psum", bufs=1, space="PSUM"))
        xq_psum = ctx.enter_context(
            tc.tile_pool(name="xq_psum", bufs=1, space="PSUM"))
        sm_psum = ctx.enter_context(
            tc.tile_pool(name="sm_psum", bufs=1, space="PSUM"))
        mp_pool = ctx.enter_context(tc.tile_pool(name="mp", bufs=16))
        pcm_pool = ctx.enter_context(tc.tile_pool(name="pcm", bufs=4))

        def sm_tile():
            """one rotating [128, 512] f32 psum bank for all small matmuls"""
            return sm_psum.tile([128, 512], F32, tag="sm", bufs=3, name="smt")

        for _rep in range(repeat):
            cc_in = [dram.tile([NCORES, M, NB], F16, name=f"cci{p}") for p in range(P)]
            cc_out = [dram.tile([M, NB], F16, name=f"cco{p}") for p in range(P)]
            xt = [const.tile([M, NB], F16, name=f"xt{p}") for p in range(P)]
            QKV = const.tile([NB, P, 3 * HK], F32, name="QKV")
            sink = const.tile([128, 4], F16, name="sink", tag="sink") if ablate else None

            # attention-chain tiles ([b, p, h, q2] layouts; contiguous X reduces)
            Q = QKV[:].rearrange("b p (t h k) -> b p t h k", t=3, k=K)
            prod = const.tile([NB, P, H, P, K], F32, name="prod")
            S = const.tile([NB, P, H, P], F32, name="S")
            mx = const.tile([NB, P, H], F32, name="mx")
            Et = const.tile([NB, P, H, P], F32, name="Et")
            den = const.tile([NB, P, H], F32, name="den")
            rden = const.tile([NB, P, H], F32, name="rden")
            ATT = const.tile([NB, P, H, P], F32, name="ATT")
            prod2 = const.tile([NB, P, H, K, P], F32, name="prod2")
            O = const.tile([NB, P, HK], F32, name="O")
            oT = [const.tile([HK, NB], F32, name=f"oT{p}") for p in range(P)]

            def attn_chain(p):
                """softmax(q.kT).v for metapath p on DVE (+ exp on ACT),
                ending with O[p] transposed into oT[p].  Issued eagerly from
                finish_p so p0/p1 run under the remaining stream."""
                ve = nc.vector
                qv = Q[:, p, 0][:, :, None, :]                 # [b, h, 1, k]
                kv = Q[:, :, 1].rearrange("b q h k -> b h q k")
                qb, kb = bass.broadcast_tensor_aps(qv, kv)
                ve.tensor_tensor(prod[:, p], qb, kb, ALU.mult)
                ve.tensor_reduce(S[:, p], prod[:, p],
                                 axis=mybir.AxisListType.X, op=ALU.add)
                ve.tensor_reduce(mx[:, p], S[:, p],
                                 axis=mybir.AxisListType.X, op=ALU.max)
                sb_, mb_ = bass.broadcast_tensor_aps(S[:, p], mx[:, p][:, :, None])
                ve.tensor_tensor(Et[:, p], sb_, mb_, ALU.subtract)
                nc.scalar.activation(Et[:, p].rearrange("b h q -> b (h q)"),
                                     Et[:, p].rearrange("b h q -> b (h q)"),
                                     AF.Exp)
                ve.tensor_reduce(den[:, p], Et[:, p],
                                 axis=mybir.AxisListType.X, op=ALU.add)
                ve.reciprocal(rden[:, p], den[:, p])
                eb_, rb_ = bass.broadcast_tensor_aps(Et[:, p], rden[:, p][:, :, None])
                ve.tensor_tensor(ATT[:, p], eb_, rb_, ALU.mult)
                av = ATT[:, p][:, :, None, :]                  # [b, h, 1, q2]
                vv = Q[:, :, 2].rearrange("b q h k -> b h k q")
                ab_, vb_ = bass.broadcast_tensor_aps(av, vv)
                ve.tensor_tensor(prod2[:, p], ab_, vb_, ALU.mult)
                ve.tensor_reduce(O[:, p].rearrange("b (h k) -> b h k", k=K),
                                 prod2[:, p],
                                 axis=mybir.AxisListType.X, op=ALU.add)
                oT_ps = sm_tile()
                nc.tensor.transpose(oT_ps[:NB, :HK], O[:, p, :], ident_f[:])
                nc.vector.tensor_copy(oT[p][:], oT_ps[:NB, :HK])

            x_ps = {}

            def x_phase(p, pas, ps):
                """psum pcmT (pass) -> f16 -> transpose -> x partial MMs."""
                cp_eng = (nc.vector, nc.scalar)[pas]
                pcmT = pcm_pool.tile([128, DP], F16, tag="pcmT")
                if cp_eng is nc.scalar:
                    nc.scalar.activation(pcmT[:], ps[:], AF.Copy)
                else:
                    nc.vector.tensor_copy(pcmT[:], ps[:])
                trp = trx_psum.tile([128, DP], F16, tag="trx")
                for dt in range(NDT):
                    nc.tensor.transpose(trp[:, dt * 128:(dt + 1) * 128],
                                        pcmT[:, dt * 128:(dt + 1) * 128],
                                        ident_h[:])
                pcm_dd = pcm_pool.tile([128, DP], F16, tag="pcmd")
                if cp_eng is nc.scalar:
                    nc.vector.tensor_copy(pcm_dd[:], trp[:])
                else:
                    nc.scalar.activation(pcm_dd[:], trp[:], AF.Copy)
                xp = x_ps[p]
                for half in range(2):
                    hb = slice(half * 512, (half + 1) * 512)
                    for dt in range(NDT):
                        nc.tensor.matmul(
                            xp[:, hb],
                            lhsT=pcm_dd[:, dt * 128:(dt + 1) * 128],
                            rhs=poolsT_sb[:, pas * NDT + dt, hb],
                            start=(pas == 0 and dt == 0), stop=False)
                    if pas == NPASS - 1:
                        # rank-2 quantization correction as 2 extra rows
                        nc.tensor.matmul(
                            xp[:, hb],
                            lhsT=corrL_sb[p][:, :],
                            rhs=corrR_sb[p][:, hb],
                            start=False, stop=True)

            def finish_p(p):
                """x psum -> sbuf f16 -> cc_in -> RS -> xt -> packed qkv."""
                xp = x_ps[p]
                xsb = const.tile([M, B], F16, name=f"xsb{p}")
                nc.vector.tensor_copy(xsb[:, 0:512], xp[:, 0:512])
                nc.scalar.activation(xsb[:, 512:B], xp[:, 512:B], AF.Copy)
                nc.gpsimd.dma_start(
                    cc_in[p].rearrange("j m b -> m j b"),
                    xsb[:].rearrange("m (j b) -> m j b", j=NCORES))
                if no_cc:
                    nc.gpsimd.dma_start(cc_out[p][:, :], cc_in[p][0, :, :])
                else:
                    nc.gpsimd.collective_compute(
                        "ReduceScatter", ALU.add,
                        replica_groups=[list(range(NCORES))],
                        ins=[cc_in[p][:, :, :]],
                        outs=[cc_out[p][:, :]],
                    )
                nc.gpsimd.dma_start(xt[p][:], cc_out[p][:, :])
                if ablate == "notail":
                    nc.vector.tensor_copy(sink[:], xt[p][:, :4])

            def qkv_chain(p):
                """packed qkv matmul for metapath p (chain runs at the tail:
                scores/values mix ALL metapaths, so it's gated on qkv(P-1))."""
                qp = sm_tile()
                nc.tensor.matmul(qp[:NB, :3 * HK], lhsT=xt[p][:], rhs=wqkv_sb[:],
                                 start=True, stop=True)
                nc.vector.tensor_add(QKV[:, p, :], qp[:NB, :3 * HK], bqkv_sb[:])
                if p == P - 1:
                    for pp in range(P):
                        attn_chain(pp)

            # stream loop: p-pass pipeline with x_phase/finish interleaved
            pend = []
            for p in range(P):
                x_ps[p] = xq_psum.tile([M, B], F32, tag="xps", name="xps")
                for pas in range(NPASS):
                    ps = mm_psum.tile([128, DP], F32, tag="mmps")
                    for g in range(NG):
                        mp_t = mp_pool.tile([128, 4, 2, DP], F8E4, tag="mpt")
                        nc.sync.dma_start(
                            mp_t[:].rearrange("q a b d -> q (a b d)"),
                            mp_d[p, pas, g, :, :])
                        for pr in range(4):
                            pair = g * 4 + pr
                            nc.tensor.matmul(
                                ps[:],
                                lhsT=pc8_sb[:, p, pair, :, :],
                                rhs=mp_t[:, pr, :, :],
                                start=(pair == 0), stop=(pair == NPAIR - 1),
                                perf_mode=DR)
                    if ablate == "nox":
                        nc.vector.tensor_copy(sink[:], ps[:, :4])
                        continue
                    # finish_p(p-1) reads the shared x psum; it must be
                    # issued before x_phase(p, 0) starts overwriting it
                    if pas == NPASS - 1 and p >= 1:
                        finish_p(p - 1)
                    # flush previous pending x-phase AFTER issuing this
                    # pass's stream so PE work interleaves
                    pend.append((p, pas, ps))
                    if len(pend) >= 2:
                        x_phase(*pend.pop(0))
                    # qkv+softmax for p-1 after x_phase's copies so its
                    # RS-wait doesn't head-block the DVE/ACT queues
                    if pas == NPASS - 1 and p >= 1 and ablate is None:
                        qkv_chain(p - 1)
            if ablate != "nox":
                while pend:
                    x_phase(*pend.pop(0))
                finish_p(P - 1)
                if ablate is None:
                    qkv_chain(P - 1)

            if ablate is not None:
                nc.gpsimd.dma_start(out_d[:, :4], sink[:NB, :])
                continue

            # ---------- tail: attended projection + pool head + output ----------
            # attendedT[m, b] = sum_p Wo.T @ O[p].T  (wo stationary)
            att_ps = sm_tile()
            for p in range(P):
                nc.tensor.matmul(att_ps[:M, :NB], lhsT=wo_sb[:], rhs=oT[p][:],
                                 start=(p == 0), stop=(p == P - 1))

            # preT (m, b) = attendedT + 3*bo ; pool head
            preT = const.tile([M, NB], F32, name="preT")
            nc.scalar.activation(preT[:], att_ps[:M, :NB], AF.Identity,
                                 bias=bo3_sb[:])
            pe_ps = sm_tile()
            nc.tensor.matmul(pe_ps[:NB, :E], lhsT=preT[:], rhs=pk_sb[:],
                             start=True, stop=True)
            pe = const.tile([NB, E], F32, name="pe")
            nc.vector.tensor_add(pe[:], pe_ps[:NB, :E], pb_sb[:])
            sq2 = const.tile([NB, E], F32, name="sq2")
            ss2 = const.tile([NB, 1], F32, name="ss2")
            nc.scalar.activation(sq2[:], pe[:], AF.Square, accum_out=ss2[:])
            # 2*sqrt(ss + EPS) via Sqrt(4*ss + 4*EPS), so npn = pe/(2*||pe||)
            nc.scalar.activation(ss2[:], ss2[:], AF.Sqrt, bias=eps_sb[:NB, :],
                                 scale=4.0)
            rr = const.tile([NB, 1], F32, name="rr")
            nc.vector.reciprocal(rr[:], ss2[:])
            npn = const.tile([NB, E], F32, name="npn")
            nc.scalar.activation(npn[:], pe[:], AF.Copy, scale=rr[:])
            npT_ps = sm_tile()
            nc.tensor.transpose(npT_ps[:E, :NB], npn[:], ident_f[:])
            nc.vector.tensor_copy(npT[:E, :], npT_ps[:E, :NB])

            # final: out = npT_aug.T @ ncT_aug = (cos+1)/2 directly.
            # out DMA stays OFF the sync queue: the next rep's mp-stream
            # DMAs live there and would head-block behind the tail.
            dq_engs = (nc.scalar, nc.gpsimd)
            for ch in range(C // 512):
                fp = sm_tile()
                nc.tensor.matmul(fp[:NB, :], lhsT=npT[:],
                                 rhs=ncT_sb[:, ch * 512:(ch + 1) * 512],
                                 start=True, stop=True)
                och = const.tile([NB, 512], F16, tag="och", bufs=6, name="och")
                if ch % 2:
                    nc.scalar.activation(och[:], fp[:NB, :], AF.Copy)
                else:
                    nc.vector.tensor_copy(och[:], fp[:NB, :])
                dq_engs[ch % 2].dma_start(out_d[:, ch * 512:(ch + 1) * 512],
                                          och[:])

    _split_multi_waits(nc)
    return nc


def _silu64(x):
    return x / (1.0 + np.exp(-x))


def _prep_inputs(inputs):
    f8 = ml_dtypes.float8_e4m3
    h16 = np.float16
    pools = np.asarray(inputs["pools"], np.float32)
    metapaths = np.asarray(inputs["metapaths"], np.float32)
    ce = np.asarray(inputs["card_embeddings"], np.float32)
    mpk = np.asarray(inputs["mp_kernels"], np.float32)
    mpb = np.asarray(inputs["mp_biases"], np.float32)

    # ---- host: pc[p] (C, M) in f64 -> e4m3 (scaled) + DoubleRow repack ----
    ce64 = ce.astype(np.float64)
    pc = np.empty((P, C, M), np.float64)
    for p in range(P):
        pc[p] = _silu64(ce64 @ mpk[p].astype(np.float64)
                        + mpb[p, :, 0].astype(np.float64)[None, :])
    pc8 = (pc * PC_SCALE).astype(f8)                      # (P, C, M)
    pcq = pc8.astype(np.float64) / PC_SCALE
    # pc8 repack: [ki, p, pair, ko, m], c = (2*pair + ko)*128 + ki
    pc8_r = (pc8.reshape(P, NPAIR, 2, 128, M)
             .transpose(3, 0, 1, 2, 4))                   # ki, p, pair, ko, m
    pc8_flat = np.ascontiguousarray(pc8_r.reshape(128, P * NPAIR * 2 * M))

    # ---- host: mp -> e4m3 ----
    mp8 = metapaths.astype(f8)                            # (P, C, C)
    mpq = mp8.astype(np.float64)

    # ---- rank-1 corrections (pc-quant and mp-quant), as outer products ----
    # x_err = sum_c dpc[c,m] * z[b,c] + sum_c pcq[c,m] * dz[b,c]
    #   dpc approx: Sdelta[m] * w[b],  w = pools @ mean_c(mp)
    #   dmp approx: u[m] * w2[b],      u = sum_c pcq,  w2 = pools @ mean_c(dmp)
    corrL = np.empty((2 * P, M), np.float64)
    corrR = np.empty((2 * P, B), np.float64)
    pools64 = pools.astype(np.float64)
    for p in range(P):
        sdelta = (pc[p] - pcq[p]).sum(axis=0)             # (M,)
        w = pools64 @ metapaths[p].astype(np.float64).mean(axis=0)   # (B,)
        u = pcq[p].sum(axis=0)                            # (M,)
        w2 = pools64 @ (metapaths[p].astype(np.float64) - mpq[p]).mean(axis=0)
        # balance the factor magnitudes for f16 storage
        for row, (lv, rv) in enumerate(((sdelta, w), (u, w2))):
            s = np.sqrt((np.abs(lv).max() + 1e-30) / (np.abs(rv).max() + 1e-30))
            corrL[2 * p + row] = lv / s
            corrR[2 * p + row] = rv * s
    corrR /= NCORES   # each core contributes corr/8; RS sums to corr

    # ---- host: normalized cards (E+1, C) fp16, ones row appended ----
    nrm = np.sqrt(np.maximum((ce.astype(np.float64) ** 2).sum(axis=1), EPS))
    ncT = (ce.astype(np.float64) / nrm[:, None]).T.astype(h16)
    ncT = np.ascontiguousarray(np.concatenate(
        [ncT, np.ones((1, C), h16)], axis=0))

    # 1/PC_SCALE folded into poolsT: x = (pc*256).T @ mp @ (pools/256)
    poolsT = np.ascontiguousarray(pools.T / PC_SCALE).astype(h16)

    # ---- packed qkv weights; 1/sqrt(K) folded into Wq (and bq) ----
    wq = np.asarray(inputs["Wq"], np.float32).reshape(M, HK) * INV_SQRT_K
    wk = np.asarray(inputs["Wk"], np.float32).reshape(M, HK)
    wv = np.asarray(inputs["Wv"], np.float32).reshape(M, HK)
    wqkv = np.concatenate([wq, wk, wv], axis=1)           # (M, 384)
    bq = np.asarray(inputs["bq"], np.float32).reshape(HK) * INV_SQRT_K
    bk = np.asarray(inputs["bk"], np.float32).reshape(HK)
    bv = np.asarray(inputs["bv"], np.float32).reshape(HK)
    bqkv = np.concatenate([bq, bk, bv])

    com = {
        "pc8": pc8_flat,
        "corrL": np.ascontiguousarray(corrL.astype(h16)),
        "ncT": ncT,
        "wqkv": np.ascontiguousarray(wqkv.astype(h16)),
        "bqkv_bc": np.ascontiguousarray(np.broadcast_to(
            bqkv.reshape(1, 3 * HK), (NB, 3 * HK)).astype(np.float32)),
        "wo": np.ascontiguousarray(np.asarray(inputs["Wo"], np.float32).reshape(HK, M)),
        "bo3_col": np.ascontiguousarray(
            (P * np.asarray(inputs["bo"], np.float32)).reshape(M, 1)),
        "pool_kernel": np.ascontiguousarray(np.asarray(inputs["pool_kernel"], np.float32)),
        "pool_bias_bc": np.ascontiguousarray(np.broadcast_to(
            np.asarray(inputs["pool_bias"], np.float32).reshape(1, E), (NB, E))),
        "ident_h": np.eye(128, dtype=h16),
        "ident_f": np.eye(128, dtype=np.float32),
    }
    in_maps = []
    for i in range(NCORES):
        m = dict(com)
        # mp stream repack: [p, pass, g, ki, pair, ko, d]
        # c = (2*(g*4+pair) + ko)*128 + ki ; d = i*D + pass*DP + dlocal
        sl = mp8[:, :, i * D:(i + 1) * D]                 # (P, C, D)
        st = sl.reshape(P, NG, 4, 2, 128, NPASS, DP)      # c-major split
        st = st.transpose(0, 5, 1, 4, 2, 3, 6)            # p, pass, g, ki, pair, ko, d
        m["mp_stream"] = np.ascontiguousarray(
            st.reshape(P, NPASS, NG, 128, 4 * 2 * DP))
        m["poolsT_shard"] = np.ascontiguousarray(poolsT[i * D:(i + 1) * D, :])
        m["corrR"] = np.ascontiguousarray(corrR.astype(h16))
        in_maps.append(m)
    return in_maps


def kernel(**inputs) -> np.ndarray:
    if "nc" not in _CACHE:
        _CACHE["nc"] = build_kernel()
    nc = _CACHE["nc"]
    in_maps = _prep_inputs(inputs)
    res = run_bass_kernel_spmd(nc, in_maps, core_ids=list(range(NCORES)))
    outs = [np.asarray(res.results[i]["out"]).astype(np.float32)
            for i in range(NCORES)]
    return np.concatenate(outs, axis=0)


if __name__ == "__main__":
    nc = build_kernel()
    print("kernel built OK")
